# revision 1
# baseline (speedup 1.0000x reference)
"""KAN encoder (2 KAN layers + relu + linear head) on 8 trn2 NeuronCores.

Strategy: data-parallel on batch (512 rows/core), all weights replicated,
feature-on-partition / batch-on-free layout throughout (no transposes).

The spline path is a dense matmul over (in*9) with B-spline bases computed
via the exact identity

    bases_k(x) * 6 = a^3 - 4*e^3,   a = relu(2 - s_k),  e = relu(a - 1),
    s_k = |2.5*x + 3.5 - k|

(the 1/6 and the spline_scaler are folded into the weights host-side; the
*4 is realized as (2e)^2 * e).

Performance notes vs the f32 baseline (720us -> 544us):
  * weights, x and all feature tensors are fp16: halves HBM traffic and
    matmuls still run at 1 cycle/row (same as f32r).
  * the basis slices share relu scale/bias, so the relu/cube chain runs as
    WIDE [128, 8*512] ops: ACT does the 8 per-k Abs + silu + packed relu,
    DVE runs packed fp16 tensor_scalar (4x_2p) / tensor_tensor (2x_1p)
    ops.  The f32 baseline was vector-bound (DVE 93% busy at 533ns per
    f32 tensor_tensor); here DVE sits at ~60% and the PE at ~93%.
  * e = relu(a-1) removes the second packed ACT relu; the *4 on e^3 is
    realized by squaring 2e, keeping everything in cheap packed DVE ops.
  * layer-0 features are computed once and kept in SBUF for both output
    halves (the baseline recomputed them); layer-1 features for the first
    8 chunks are computed during og1's matmuls so layer 1 starts hot.
"""
import numpy as np
from contextlib import ExitStack

from concourse import bacc, tile, mybir
from concourse.bass_utils import run_bass_kernel_spmd

F32 = mybir.dt.float32
F16 = mybir.dt.float16
AF = mybir.ActivationFunctionType
ALU = mybir.AluOpType

B, D_IN, H0, H1, L = 4096, 1024, 2048, 1024, 512
NCORES = 8
BC = B // NCORES          # 512 batch cols per core
NB = 512                  # free-dim (batch) tile = full per-core batch
CBRT4 = float(4.0 ** (1.0 / 3.0))

_cache = {}


def _build_full():
    nc = bacc.Bacc("TRN2", target_bir_lowering=False, debug=False,
                   num_devices=NCORES)

    x_d = nc.dram_tensor("x_d", [8, 128, BC], F16, kind="ExternalInput")
    w0_d = nc.dram_tensor("w0_d", [8, 2, 2, 128, 9 * NB], F16,
                          kind="ExternalInput")
    w1_d = nc.dram_tensor("w1_d", [16, 2, 128, 9 * NB], F16,
                          kind="ExternalInput")
    dw_d = nc.dram_tensor("dw_d", [8, 128, L], F16, kind="ExternalInput")
    db_d = nc.dram_tensor("db_d", [128, 4], F32, kind="ExternalInput")
    o_d = nc.dram_tensor("o_d", [4, 128, BC], F32, kind="ExternalOutput")

    with tile.TileContext(nc) as tc, ExitStack() as ctx:
        psum = ctx.enter_context(tc.tile_pool(name="psum", bufs=1,
                                              space="PSUM"))
        fpool = ctx.enter_context(tc.tile_pool(name="fpool", bufs=1))
        wpool = ctx.enter_context(tc.tile_pool(name="wpool", bufs=1))
        ipool = ctx.enter_context(tc.tile_pool(name="ipool", bufs=1))
        opool = ctx.enter_context(tc.tile_pool(name="opool", bufs=1))

        fics = [None] * 8     # current feats tile per contraction chunk
        h0ts = [None] * 16    # layer-0 output chunks

        bias_tiles = {}

        def bias_ap(val):
            val = float(val)
            if val not in bias_tiles:
                t = opool.tile([128, 1], F32, tag=f"b{len(bias_tiles)}",
                               name=f"bias{len(bias_tiles)}")
                nc.gpsimd.memset(t[:, :], val)
                bias_tiles[val] = t
            return bias_tiles[val][:, :]

        def emit_feats(src_ap, fic, tagp):
            """fic (128, 9*NB) f16: j=0 silu(src); j=1+k -> 6*bases_k(src)."""
            sp = ipool.tile([128, 8 * NB], F16, tag="sp", bufs=2,
                            name=f"sp{tagp}")
            av = ipool.tile([128, 8 * NB], F16, tag="av", bufs=2,
                            name=f"av{tagp}")
            et = ipool.tile([128, 8 * NB], F16, tag="et", bufs=1,
                            name=f"et{tagp}")
            nc.scalar.activation(fic[:, 0:NB], src_ap, AF.Silu,
                                 bias=bias_ap(0.0), scale=1.0)
            for k in range(8):
                nc.scalar.activation(sp[:, k * NB:(k + 1) * NB], src_ap,
                                     AF.Abs, bias=bias_ap(3.5 - k), scale=2.5)
            nc.scalar.activation(av[:, :], sp[:, :], AF.Relu,
                                 bias=bias_ap(2.0), scale=-1.0)
            nc.vector.tensor_scalar(et[:, :], av[:, :], 1.0, 0.0,
                                    ALU.subtract, ALU.max)
            a2 = ipool.tile([128, 8 * NB], F16, tag="p", bufs=1,
                            name=f"a2{tagp}")
            a3 = ipool.tile([128, 8 * NB], F16, tag="q", bufs=1,
                            name=f"a3{tagp}")
            nc.vector.tensor_tensor(a2[:, :], av[:, :], av[:, :], ALU.mult)
            nc.vector.tensor_tensor(a3[:, :], a2[:, :], av[:, :], ALU.mult)
            gt = ipool.tile([128, 8 * NB], F16, tag="p", bufs=1,
                            name=f"gt{tagp}")
            nc.vector.tensor_scalar(gt[:, :], et[:, :], 2.0, None, ALU.mult)
            e2 = ipool.tile([128, 8 * NB], F16, tag="r", bufs=1,
                            name=f"e2{tagp}")
            nc.vector.tensor_tensor(e2[:, :], gt[:, :], gt[:, :], ALU.mult)
            e3 = ipool.tile([128, 8 * NB], F16, tag="p", bufs=1,
                            name=f"e3{tagp}")
            nc.vector.tensor_tensor(e3[:, :], e2[:, :], et[:, :], ALU.mult)
            nc.vector.tensor_tensor(fic[:, NB:9 * NB], a3[:, :], e3[:, :],
                                    ALU.subtract)

        def kan_matmuls(pts, wsl, fic, ic, n_ic):
            for half in range(2):
                for oc4 in range(4):
                    oc = half * 4 + oc4
                    base = oc4 * 128
                    for j in range(9):
                        nc.tensor.matmul(
                            pts[oc][:, :],
                            wsl[half][:, j * NB + base:j * NB + base + 128],
                            fic[:, j * NB:(j + 1) * NB],
                            start=(ic == 0 and j == 0),
                            stop=(ic == n_ic - 1 and j == 8))

        # ---- Layer 0: out split in two groups of 8 PSUM banks ----
        for og in range(2):
            pts = [psum.tile([128, NB], F32, tag=f"bank{oc}",
                             name=f"psA{og}_{oc}") for oc in range(8)]
            for ic in range(8):
                if og == 0:
                    xt = ipool.tile([128, NB], F16, tag="xt", bufs=2,
                                    name=f"xt{ic}")
                    nc.sync.dma_start(out=xt[:, :], in_=x_d[ic, :, :])
                    fic = fpool.tile([128, 9 * NB], F16, tag=f"fic{ic}",
                                     name=f"f0_{ic}")
                    emit_feats(xt[:, :], fic, f"0_{ic}")
                    fics[ic] = fic
                wsl = []
                for half in range(2):
                    wt = wpool.tile([128, 9 * NB], F16, tag="w", bufs=3,
                                    name=f"w0_{og}_{ic}_{half}")
                    nc.sync.dma_start(out=wt[:, :], in_=w0_d[ic, og, half])
                    wsl.append(wt)
                kan_matmuls(pts, wsl, fics[ic], ic, 8)
                if og == 1:
                    # recompute feats for layer 1 while og1 matmuls drain
                    fic = fpool.tile([128, 9 * NB], F16, tag=f"fic{ic}",
                                     name=f"f1_{ic}")
                    emit_feats(h0ts[ic][:, :], fic, f"1_{ic}")
                    fics[ic] = fic
            for oc in range(8):
                h0t = fpool.tile([128, NB], F16, tag=f"h0_{og * 8 + oc}",
                                 name=f"h0t{og * 8 + oc}")
                nc.scalar.activation(h0t[:, :], pts[oc][:, :], AF.Copy,
                                     bias=0.0, scale=1.0)
                h0ts[og * 8 + oc] = h0t

        # head weights: small, load while layer 1 runs
        dwt = opool.tile([128, 8, L], F16, name="dwt")
        for ic in range(8):
            nc.sync.dma_start(out=dwt[:, ic, :], in_=dw_d[ic, :, :])
        dbt = opool.tile([128, 4], F32, name="dbt")
        nc.sync.dma_start(out=dbt[:, :], in_=db_d[:, :])

        # ---- Layer 1: 8 out chunks, 16 contraction chunks ----
        pts = [psum.tile([128, NB], F32, tag=f"bank{oc}", name=f"psB{oc}")
               for oc in range(8)]
        for ic in range(16):
            if ic >= 8:
                fic = fpool.tile([128, 9 * NB], F16, tag=f"fic{ic - 8}",
                                 name=f"f1_{ic}")
                emit_feats(h0ts[ic][:, :], fic, f"1_{ic}")
                fics[ic - 8] = fic
            wsl = []
            for half in range(2):
                wt = wpool.tile([128, 9 * NB], F16, tag="w", bufs=3,
                                name=f"w1_{ic}_{half}")
                nc.sync.dma_start(out=wt[:, :], in_=w1_d[ic, half])
                wsl.append(wt)
            kan_matmuls(pts, wsl, fics[ic % 8], ic, 16)

        # ---- Head: relu(h1) @ dw.T + db ----
        rl = opool.tile([128, 8 * NB], F16, name="rl")
        for oc in range(8):
            nc.scalar.activation(rl[:, oc * NB:(oc + 1) * NB],
                                 pts[oc][:, :], AF.Relu,
                                 bias=bias_ap(0.0), scale=1.0)
        for lc in range(4):
            pt = psum.tile([128, NB], F32, tag=f"bank{lc}", name=f"psC{lc}")
            for ic in range(8):
                nc.tensor.matmul(pt[:, :], dwt[:, ic, lc * 128:(lc + 1) * 128],
                                 rl[:, ic * NB:(ic + 1) * NB],
                                 start=(ic == 0), stop=(ic == 7))
            ot = opool.tile([128, NB], F32, tag="ot", bufs=2, name=f"ot{lc}")
            nc.scalar.activation(ot[:, :], pt[:, :], AF.Identity,
                                 bias=dbt[:, lc:lc + 1], scale=1.0)
            nc.sync.dma_start(out=o_d[lc, :, :], in_=ot[:, :])

    nc.compile()
    return nc


def _prep_weights(bw0, sw0, ss0, bw1, sw1, ss1, dw, db):
    # KAN layer weights: slot j=0 -> base weight, j=1+k -> sw*ss/6
    W0 = np.empty((D_IN, 9, H0), np.float32)
    W0[:, 0, :] = bw0.T
    W0[:, 1:, :] = (sw0 * (ss0[:, :, None] / 6.0)).transpose(1, 2, 0)
    w0 = np.ascontiguousarray(
        W0.reshape(8, 128, 9, 2, 2, 4, 128)
          .transpose(0, 3, 4, 1, 2, 5, 6)
          .reshape(8, 2, 2, 128, 9 * NB)).astype(np.float16)
    W1 = np.empty((H0, 9, H1), np.float32)
    W1[:, 0, :] = bw1.T
    W1[:, 1:, :] = (sw1 * (ss1[:, :, None] / 6.0)).transpose(1, 2, 0)
    w1 = np.ascontiguousarray(
        W1.reshape(16, 128, 9, 2, 4, 128)
          .transpose(0, 3, 1, 2, 4, 5)
          .reshape(16, 2, 128, 9 * NB)).astype(np.float16)
    dwt = np.ascontiguousarray(dw.T.reshape(8, 128, L)).astype(np.float16)
    dbt = np.ascontiguousarray(db.reshape(4, 128).T.astype(np.float32))
    return w0, w1, dwt, dbt


def kernel(x, bw0, sw0, ss0, bw1, sw1, ss1, dw, db):
    if "nc" not in _cache:
        _cache["nc"] = _build_full()
    nc = _cache["nc"]
    w0, w1, dwt, dbt = _prep_weights(
        np.asarray(bw0, np.float32), np.asarray(sw0, np.float32),
        np.asarray(ss0, np.float32), np.asarray(bw1, np.float32),
        np.asarray(sw1, np.float32), np.asarray(ss1, np.float32),
        np.asarray(dw, np.float32), np.asarray(db, np.float32))
    xT = np.ascontiguousarray(np.asarray(x, np.float32).T).astype(np.float16)
    in_maps = []
    for c in range(NCORES):
        xc = np.ascontiguousarray(
            xT[:, c * BC:(c + 1) * BC].reshape(8, 128, BC))
        in_maps.append({"x_d": xc, "w0_d": w0, "w1_d": w1,
                        "dw_d": dwt, "db_d": dbt})
    _cache["in_maps"] = in_maps
    res = run_bass_kernel_spmd(nc, in_maps, list(range(NCORES)))
    out = np.empty((B, L), np.float32)
    for c in range(NCORES):
        oc = res.results[c]["o_d"]          # (4, 128, BC)
        out[c * BC:(c + 1) * BC, :] = oc.reshape(L, BC).T
    return out



# revision 8
# speedup vs baseline: 1.6724x; 1.6724x over previous
"""KAN encoder (2 KAN layers + relu + linear head) on 8 trn2 NeuronCores.

Strategy: data-parallel on batch (512 rows/core), all weights replicated,
feature-on-partition / batch-on-free layout throughout (no transposes).

The spline path is a dense contraction over (in, 8 basis slots) computed
with fp8e4 DoubleRow matmuls (two basis slots per PE pass), via the exact
identity

    6*bases_k(x) = a^3 - 4*e^3,  a = relu(2 - s_k), e = relu(1 - s_k),
    s_k = |2.5*x + 3.5 - k|

The kernel stores the NEGATED feature  spl_k = -(a^3 - (c*e)^3), c=4^(1/3)
in fp8e4 and the host negates/scales the spline weights to compensate:
W8 = -(sw*ss/6) * S, with S a runtime power of two sized so W8 sits in
e4m3's normal range; the 1/S descale rides the psum->sbuf copy as a
per-partition scale AP.  The base path (SiLU @ bw.T) and head stay fp16
(their contributions are ~30x larger, fp8 there would blow the error
budget; the spline path's contribution is small so fp8e4 costs ~nothing).

Elementwise basis work is spread over three engines per 128-feature chunk:
  DVE : u_k = 2.5x+3.5-k (8 narrow), st=min(|u|,2), na=st-2 (=-a),
        a3m=a2*na (=-a^3), spl=a3m+e3 (fp8 out)
  ACT : silu, ev=relu(c-c*st) (=c*e), a2=Square(na)
  GPS : e2=ev*ev, e3=e2*ev (=4e^3)
"""
import numpy as np
import ml_dtypes
from contextlib import ExitStack

from concourse import bacc, tile, mybir
from concourse.bass_utils import run_bass_kernel_spmd

import concourse.dve_ops as _dvo
from concourse.dve_ops import DveOp as _DveOp
from concourse.dve_spec import (Spec as _Spec, Src0 as _Src0, Src1 as _Src1,
                                C0 as _C0, relu as _relu, minn as _minn,
                                lower as _lower)
from concourse.dve_uop import DveOpSpec as _DveOpSpec

_KANB_NAME = "KAN_BSPLINE_ANT"


def _register_kanb():
    """Custom DVE op: out = relu(in1)^3 - (s0 - min(in0, s0))^3.

    With in0 = s = |2.5x+3.5-k|, in1 = C*(1-s) (C=4^(1/3)), s0 = 2.0 this
    is 4e^3 - a^3 = -(6*B_k(x)) -- the whole cubic B-spline bump in one
    8-stage DVE pass (fp8 output costs nothing extra).
    """
    if _KANB_NAME in _dvo._SUB_OPCODE_FOR_NAME:
        return next(op for op in _dvo.OPS if op.name == _KANB_NAME)
    c = _minn(_Src0, _C0)
    a = _C0 - c
    g = _relu(_Src1)

    def _ref(in0, in1, s0):
        aa = s0 - np.minimum(np.asarray(in0, np.float32), s0)
        gg = np.maximum(np.asarray(in1, np.float32), 0.0)
        return gg * gg * gg - aa * aa * aa

    spec = _Spec(body=g * g * g - a * a * a, reference=_ref)
    row = _dvo._CUSTOM_DVE_ROW_BASE + len(_dvo.OPS)
    assert row < 0x20
    _dvo._SUB_OPCODE_FOR_NAME[_KANB_NAME] = row
    shas = {}
    for ver in ("v3", "v4"):
        s = _DveOpSpec(name=_KANB_NAME, opcode=row,
                       uops=_lower(spec, ver=ver), rd1_en=True)
        shas[ver] = s.sha(ver)
    op = _DveOp(_KANB_NAME, spec, subdim=False, uops_sha=shas)
    _dvo.OPS.append(op)
    _dvo.CUSTOM_DVE_SPECS[_KANB_NAME] = spec
    return op


KANB = _register_kanb()

F32 = mybir.dt.float32
F16 = mybir.dt.float16
F8 = mybir.dt.float8e4
AF = mybir.ActivationFunctionType
ALU = mybir.AluOpType
DR = mybir.MatmulPerfMode.DoubleRow

B, D_IN, H0, H1, L = 4096, 1024, 2048, 1024, 512
NCORES = 8
BC = B // NCORES          # 512 batch cols per core
NB = 512                  # free-dim (batch) tile = full per-core batch
CBRT4 = float(4.0 ** (1.0 / 3.0))

_cache = {}


def _build_full():
    nc = bacc.Bacc("TRN2", target_bir_lowering=False, debug=False,
                   num_devices=NCORES)

    x_d = nc.dram_tensor("x_d", [8, 128, BC], F16, kind="ExternalInput")
    w0b_d = nc.dram_tensor("w0b_d", [8, 2, 128, 8, 128], F16,
                           kind="ExternalInput")
    w0s_d = nc.dram_tensor("w0s_d", [8, 2, 128, 8, 4, 2, 128], F8,
                           kind="ExternalInput")
    w1b_d = nc.dram_tensor("w1b_d", [16, 128, 8, 128], F16,
                           kind="ExternalInput")
    w1s_d = nc.dram_tensor("w1s_d", [16, 128, 8, 4, 2, 128], F8,
                           kind="ExternalInput")
    dw_d = nc.dram_tensor("dw_d", [8, 128, L], F16, kind="ExternalInput")
    db_d = nc.dram_tensor("db_d", [128, 4], F32, kind="ExternalInput")
    sc_d = nc.dram_tensor("sc_d", [128, 2], F32, kind="ExternalInput")
    o_d = nc.dram_tensor("o_d", [4, 128, BC], F32, kind="ExternalOutput")

    with tile.TileContext(nc) as tc, ExitStack() as ctx:
        psum = ctx.enter_context(tc.tile_pool(name="psum", bufs=1,
                                              space="PSUM"))
        fpool = ctx.enter_context(tc.tile_pool(name="fpool", bufs=1))
        wpool = ctx.enter_context(tc.tile_pool(name="wpool", bufs=1))
        ipool = ctx.enter_context(tc.tile_pool(name="ipool", bufs=1))
        opool = ctx.enter_context(tc.tile_pool(name="opool", bufs=1))

        sct = opool.tile([128, 2], F32, name="sct")
        nc.sync.dma_start(out=sct[:, :], in_=sc_d[:, :])

        bfs = [None] * 8      # base (silu) feature tiles per chunk
        spls = [None] * 8     # fp8 spline feature tiles per chunk
        h0ts = [None] * 16    # layer-0 output chunks

        bias_tiles = {}

        def bias_ap(val):
            val = float(val)
            if val not in bias_tiles:
                t = opool.tile([128, 1], F32, tag=f"b{len(bias_tiles)}",
                               name=f"bias{len(bias_tiles)}")
                nc.gpsimd.memset(t[:, :], val)
                bias_tiles[val] = t
            return bias_tiles[val][:, :]

        def emit_feats(src_ap, m, tagp):
            """src [128,NB] f16 -> bf f16 [128,NB], spl fp8 [128,8,NB].

            spl_k = 4e^3 - a^3 = -(6*B_k): ACT makes s_k = |2.5x+3.5-k|,
            one DVE ts makes v = C*(1-s), the custom DVE op does the rest
            (a=relu(2-s) via min/sub, g=relu(v), out = g^3 - a^3) with fp8
            output for free.
            """
            bf = fpool.tile([128, NB], F16, tag=f"bf{m}", name=f"bf{tagp}")
            spl = fpool.tile([128, 8, NB], F8, tag=f"spl{m}",
                             name=f"spl{tagp}")
            nc.scalar.activation(bf[:, :], src_ap, AF.Silu,
                                 bias=bias_ap(0.0), scale=1.0)
            sp = ipool.tile([128, 8, NB], F16, tag="sp", bufs=2,
                            name=f"sp{tagp}")
            for k in range(8):
                nc.scalar.activation(sp[:, k, :], src_ap, AF.Abs,
                                     bias=bias_ap(3.5 - k), scale=2.5)
            v = ipool.tile([128, 8, NB], F16, tag="v", bufs=2,
                           name=f"v{tagp}")
            nc.vector.tensor_scalar(v[:, :, :], sp[:, :, :], -CBRT4, CBRT4,
                                    ALU.mult, ALU.add)
            nc.vector._custom_dve(KANB, out=spl[:, :, :], in0=sp[:, :, :],
                                  in1=v[:, :, :], s0=2.0)
            return bf, spl

        def kan_matmuls(pts, wbt, wst, bf, spl, ic, n_ic):
            for oc in range(8):
                nc.tensor.matmul(
                    pts[oc][:, :], wbt[:, oc, :], bf[:, :],
                    start=(ic == 0), stop=False)
                for p in range(4):
                    nc.tensor.matmul(
                        pts[oc][:, :], wst[:, oc, p, :, :],
                        spl[:, 2 * p:2 * p + 2, :],
                        start=False,
                        stop=(ic == n_ic - 1 and p == 3),
                        perf_mode=DR)

        # ---- Layer 0: out split in two groups of 8 PSUM banks ----
        for og in range(2):
            pts = [psum.tile([128, NB], F32, tag=f"bank{oc}",
                             name=f"psA{og}_{oc}") for oc in range(8)]
            for ic in range(8):
                if og == 0:
                    xt = ipool.tile([128, NB], F16, tag="xt", bufs=2,
                                    name=f"xt{ic}")
                    nc.sync.dma_start(out=xt[:, :], in_=x_d[ic, :, :])
                    bfs[ic], spls[ic] = emit_feats(xt[:, :], ic, f"0_{ic}")
                wbt = wpool.tile([128, 8, 128], F16, tag="wb", bufs=3,
                                 name=f"w0b_{og}_{ic}")
                nc.sync.dma_start(out=wbt[:, :, :], in_=w0b_d[ic, og])
                wst = wpool.tile([128, 8, 4, 2, 128], F8, tag="ws", bufs=3,
                                 name=f"w0s_{og}_{ic}")
                nc.sync.dma_start(out=wst[:, :, :, :, :], in_=w0s_d[ic, og])
                kan_matmuls(pts, wbt, wst, bfs[ic], spls[ic], ic, 8)
                if og == 1:
                    # recompute feats for layer 1 while og1 matmuls drain
                    bfs[ic], spls[ic] = emit_feats(h0ts[ic][:, :], ic,
                                                   f"1_{ic}")
            for oc in range(8):
                h0t = fpool.tile([128, NB], F16, tag=f"h0_{og * 8 + oc}",
                                 name=f"h0t{og * 8 + oc}")
                nc.scalar.activation(h0t[:, :], pts[oc][:, :], AF.Copy,
                                     bias=0.0, scale=sct[:, 0:1])
                h0ts[og * 8 + oc] = h0t

        # head weights: small, load while layer 1 runs
        dwt = opool.tile([128, 8, L], F16, name="dwt")
        for ic in range(8):
            nc.sync.dma_start(out=dwt[:, ic, :], in_=dw_d[ic, :, :])
        dbt = opool.tile([128, 4], F32, name="dbt")
        nc.sync.dma_start(out=dbt[:, :], in_=db_d[:, :])

        # ---- Layer 1: 8 out chunks, 16 contraction chunks ----
        pts = [psum.tile([128, NB], F32, tag=f"bank{oc}", name=f"psB{oc}")
               for oc in range(8)]
        for ic in range(16):
            if ic >= 8:
                bfs[ic % 8], spls[ic % 8] = emit_feats(
                    h0ts[ic][:, :], ic % 8, f"1_{ic}")
            wbt = wpool.tile([128, 8, 128], F16, tag="wb", bufs=3,
                             name=f"w1b_{ic}")
            nc.sync.dma_start(out=wbt[:, :, :], in_=w1b_d[ic])
            wst = wpool.tile([128, 8, 4, 2, 128], F8, tag="ws", bufs=3,
                             name=f"w1s_{ic}")
            nc.sync.dma_start(out=wst[:, :, :, :, :], in_=w1s_d[ic])
            kan_matmuls(pts, wbt, wst, bfs[ic % 8], spls[ic % 8], ic, 16)

        # ---- Head: relu(h1) @ dw.T + db ----
        rl = opool.tile([128, 8, NB], F16, name="rl")
        for oc in range(8):
            nc.scalar.activation(rl[:, oc, :], pts[oc][:, :], AF.Relu,
                                 bias=bias_ap(0.0), scale=sct[:, 1:2])
        for lc in range(4):
            pt = psum.tile([128, NB], F32, tag=f"bank{lc}", name=f"psC{lc}")
            for ic in range(8):
                nc.tensor.matmul(pt[:, :], dwt[:, ic, lc * 128:(lc + 1) * 128],
                                 rl[:, ic, :],
                                 start=(ic == 0), stop=(ic == 7))
            ot = opool.tile([128, NB], F32, tag="ot", bufs=2, name=f"ot{lc}")
            nc.scalar.activation(ot[:, :], pt[:, :], AF.Identity,
                                 bias=dbt[:, lc:lc + 1], scale=1.0)
            nc.sync.dma_start(out=o_d[lc, :, :], in_=ot[:, :])

    nc.compile()
    return nc


def _pow2_scale(maxabs):
    # largest power of two keeping maxabs*S <= ~120 (half of e4m3 top 240)
    if not np.isfinite(maxabs) or maxabs <= 0:
        return 1.0
    return float(2.0 ** np.floor(np.log2(120.0 / maxabs)))


def _prep_weights(bw0, sw0, ss0, bw1, sw1, ss1, dw, db):
    out = {}
    for li, (bw, sw, ss, n_ic, n_og) in enumerate(
            ((bw0, sw0, ss0, 8, 2), (bw1, sw1, ss1, 16, 1))):
        W = -(sw * (ss[:, :, None] / 6.0))          # [out, in, 8]
        S = _pow2_scale(float(np.abs(W).max()))
        out[f"sc{li}"] = 1.0 / S
        W8 = np.clip(W * S, -240.0, 240.0).astype(ml_dtypes.float8_e4m3)
        n_out, n_in = W.shape[0], W.shape[1]
        # [out, in, k] -> [ic, og, isub, oc, p, two, osub]
        v = W8.reshape(n_og, 8, 128, n_ic, 128, 4, 2)
        ws = np.ascontiguousarray(v.transpose(3, 0, 4, 1, 5, 6, 2))
        out[f"w{li}s"] = ws.reshape((n_ic,) + ((2,) if n_og == 2 else ())
                                    + (128, 8, 4, 2, 128))
        Wb = (bw * S).astype(np.float16)
        vb = Wb.reshape(n_og, 8, 128, n_ic, 128)
        wb = np.ascontiguousarray(vb.transpose(3, 0, 4, 1, 2))
        out[f"w{li}b"] = wb.reshape((n_ic,) + ((2,) if n_og == 2 else ())
                                    + (128, 8, 128))
    dwt = np.ascontiguousarray(dw.T.reshape(8, 128, L)).astype(np.float16)
    dbt = np.ascontiguousarray(db.reshape(4, 128).T.astype(np.float32))
    sct = np.empty((128, 2), np.float32)
    sct[:, 0] = out["sc0"]
    sct[:, 1] = out["sc1"]
    return out["w0b"], out["w0s"], out["w1b"], out["w1s"], dwt, dbt, sct


def kernel(x, bw0, sw0, ss0, bw1, sw1, ss1, dw, db):
    if "nc" not in _cache:
        _cache["nc"] = _build_full()
    nc = _cache["nc"]
    w0b, w0s, w1b, w1s, dwt, dbt, sct = _prep_weights(
        np.asarray(bw0, np.float32), np.asarray(sw0, np.float32),
        np.asarray(ss0, np.float32), np.asarray(bw1, np.float32),
        np.asarray(sw1, np.float32), np.asarray(ss1, np.float32),
        np.asarray(dw, np.float32), np.asarray(db, np.float32))
    xT = np.ascontiguousarray(np.asarray(x, np.float32).T).astype(np.float16)
    in_maps = []
    for c in range(NCORES):
        xc = np.ascontiguousarray(
            xT[:, c * BC:(c + 1) * BC].reshape(8, 128, BC))
        in_maps.append({"x_d": xc, "w0b_d": w0b, "w0s_d": w0s,
                        "w1b_d": w1b, "w1s_d": w1s,
                        "dw_d": dwt, "db_d": dbt, "sc_d": sct})
    _cache["in_maps"] = in_maps
    res = run_bass_kernel_spmd(nc, in_maps, list(range(NCORES)))
    out = np.empty((B, L), np.float32)
    for c in range(NCORES):
        oc = res.results[c]["o_d"]          # (4, 128, BC)
        out[c * BC:(c + 1) * BC, :] = oc.reshape(L, BC).T
    return out


# revision 13
# speedup vs baseline: 1.7678x; 1.0570x over previous
"""KAN encoder (2 KAN layers + relu + linear head) on 8 trn2 NeuronCores.

Strategy: data-parallel on batch (512 rows/core), all weights replicated,
feature-on-partition / batch-on-free layout throughout (no transposes).

The spline path is a dense contraction over (in, 8 basis slots) computed
with fp8e4 DoubleRow matmuls (two basis slots per PE pass), via the exact
identity

    6*bases_k(x) = a^3 - 4*e^3,  a = relu(2 - s_k), e = relu(1 - s_k),
    s_k = |2.5*x + 3.5 - k|

The kernel stores the NEGATED feature  spl_k = -(a^3 - (c*e)^3), c=4^(1/3)
in fp8e4 and the host negates/scales the spline weights to compensate:
W8 = -(sw*ss/6) * S, with S a runtime power of two sized so W8 sits in
e4m3's normal range; the 1/S descale rides the psum->sbuf copy as a
per-partition scale AP.  The base path (SiLU @ bw.T) and head stay fp16
(their contributions are ~30x larger, fp8 there would blow the error
budget; the spline path's contribution is small so fp8e4 costs ~nothing).

Elementwise basis work is spread over three engines per 128-feature chunk:
  DVE : u_k = 2.5x+3.5-k (8 narrow), st=min(|u|,2), na=st-2 (=-a),
        a3m=a2*na (=-a^3), spl=a3m+e3 (fp8 out)
  ACT : silu, ev=relu(c-c*st) (=c*e), a2=Square(na)
  GPS : e2=ev*ev, e3=e2*ev (=4e^3)
"""
import numpy as np
import ml_dtypes
from contextlib import ExitStack

from concourse import bacc, tile, mybir
from concourse.bass_utils import run_bass_kernel_spmd

import concourse.dve_ops as _dvo
from concourse.dve_ops import DveOp as _DveOp
from concourse.dve_spec import (Spec as _Spec, Src0 as _Src0, Src1 as _Src1,
                                C0 as _C0, relu as _relu, minn as _minn,
                                lower as _lower)
from concourse.dve_uop import DveOpSpec as _DveOpSpec

_KANB_NAME = "KAN_BSPLINE_ANT"


def _register_kanb():
    """Custom DVE op: out = relu(in1)^3 - (s0 - min(in0, s0))^3.

    With in0 = s = |2.5x+3.5-k|, in1 = C*(1-s) (C=4^(1/3)), s0 = 2.0 this
    is 4e^3 - a^3 = -(6*B_k(x)) -- the whole cubic B-spline bump in one
    8-stage DVE pass (fp8 output costs nothing extra).
    """
    if _KANB_NAME in _dvo._SUB_OPCODE_FOR_NAME:
        return next(op for op in _dvo.OPS if op.name == _KANB_NAME)
    c = _minn(_Src0, _C0)
    a = _C0 - c
    g = _relu(_Src1)

    def _ref(in0, in1, s0):
        aa = s0 - np.minimum(np.asarray(in0, np.float32), s0)
        gg = np.maximum(np.asarray(in1, np.float32), 0.0)
        return gg * gg * gg - aa * aa * aa

    spec = _Spec(body=g * g * g - a * a * a, reference=_ref)
    row = _dvo._CUSTOM_DVE_ROW_BASE + len(_dvo.OPS)
    assert row < 0x20
    _dvo._SUB_OPCODE_FOR_NAME[_KANB_NAME] = row
    shas = {}
    for ver in ("v3", "v4"):
        s = _DveOpSpec(name=_KANB_NAME, opcode=row,
                       uops=_lower(spec, ver=ver), rd1_en=True)
        shas[ver] = s.sha(ver)
    op = _DveOp(_KANB_NAME, spec, subdim=False, uops_sha=shas)
    _dvo.OPS.append(op)
    _dvo.CUSTOM_DVE_SPECS[_KANB_NAME] = spec
    return op


KANB = _register_kanb()

F32 = mybir.dt.float32
F16 = mybir.dt.float16
F8 = mybir.dt.float8e4
AF = mybir.ActivationFunctionType
ALU = mybir.AluOpType
DR = mybir.MatmulPerfMode.DoubleRow

B, D_IN, H0, H1, L = 4096, 1024, 2048, 1024, 512
NCORES = 8
BC = B // NCORES          # 512 batch cols per core
NB = 512                  # free-dim (batch) tile = full per-core batch
CBRT4 = float(4.0 ** (1.0 / 3.0))

_cache = {}


def _build_full():
    nc = bacc.Bacc("TRN2", target_bir_lowering=False, debug=False,
                   num_devices=NCORES)

    x_d = nc.dram_tensor("x_d", [8, 128, BC], F16, kind="ExternalInput")
    w0b_d = nc.dram_tensor("w0b_d", [8, 2, 128, 8, 128], F16,
                           kind="ExternalInput")
    w0s_d = nc.dram_tensor("w0s_d", [8, 2, 128, 8, 4, 2, 128], F8,
                           kind="ExternalInput")
    w1b_d = nc.dram_tensor("w1b_d", [16, 128, 8, 128], F16,
                           kind="ExternalInput")
    w1s_d = nc.dram_tensor("w1s_d", [16, 128, 8, 4, 2, 128], F8,
                           kind="ExternalInput")
    dw_d = nc.dram_tensor("dw_d", [8, 128, L], F16, kind="ExternalInput")
    db_d = nc.dram_tensor("db_d", [128, 4], F32, kind="ExternalInput")
    sc_d = nc.dram_tensor("sc_d", [128, 2], F32, kind="ExternalInput")
    o_d = nc.dram_tensor("o_d", [4, 128, BC], F32, kind="ExternalOutput")

    with tile.TileContext(nc) as tc, ExitStack() as ctx:
        psum = ctx.enter_context(tc.tile_pool(name="psum", bufs=1,
                                              space="PSUM"))
        fpool = ctx.enter_context(tc.tile_pool(name="fpool", bufs=1))
        wpool = ctx.enter_context(tc.tile_pool(name="wpool", bufs=1))
        ipool = ctx.enter_context(tc.tile_pool(name="ipool", bufs=1))
        opool = ctx.enter_context(tc.tile_pool(name="opool", bufs=1))

        sct = opool.tile([128, 2], F32, name="sct")
        nc.sync.dma_start(out=sct[:, :], in_=sc_d[:, :])

        bfs = [None] * 8      # base (silu) feature tiles per chunk
        spls = [None] * 8     # fp8 spline feature tiles per chunk
        h0ts = [None] * 16    # layer-0 output chunks

        bias_tiles = {}

        def bias_ap(val):
            val = float(val)
            if val not in bias_tiles:
                t = opool.tile([128, 1], F32, tag=f"b{len(bias_tiles)}",
                               name=f"bias{len(bias_tiles)}")
                nc.gpsimd.memset(t[:, :], val)
                bias_tiles[val] = t
            return bias_tiles[val][:, :]

        def emit_spline(src_ap, m, tagp):
            """src [128,NB] f16 -> spl fp8 [128,8,NB].

            spl_k = 4e^3 - a^3 = -(6*B_k): ACT makes s_k = |2.5x+3.5-k|,
            one DVE ts makes v = C*(1-s), the custom DVE op does the rest
            (a=relu(2-s) via min/sub, g=relu(v), out = g^3 - a^3) with fp8
            output for free.
            """
            spl = fpool.tile([128, 8, NB], F8, tag=f"spl{m}",
                             name=f"spl{tagp}")
            sp = ipool.tile([128, 8, NB], F16, tag="sp", bufs=2,
                            name=f"sp{tagp}")
            for k in range(8):
                nc.scalar.activation(sp[:, k, :], src_ap, AF.Abs,
                                     bias=bias_ap(3.5 - k), scale=2.5)
            v = ipool.tile([128, 8, NB], F16, tag="v", bufs=2,
                           name=f"v{tagp}")
            nc.vector.tensor_scalar(v[:, :, :], sp[:, :, :], -CBRT4, CBRT4,
                                    ALU.mult, ALU.add)
            nc.vector._custom_dve(KANB, out=spl[:, :, :], in0=sp[:, :, :],
                                  in1=v[:, :, :], s0=2.0)
            return spl

        def emit_feats(src_ap, m, tagp):
            bf = fpool.tile([128, NB], F16, tag=f"bf{m}", name=f"bf{tagp}")
            nc.scalar.activation(bf[:, :], src_ap, AF.Silu,
                                 bias=bias_ap(0.0), scale=1.0)
            return bf, emit_spline(src_ap, m, tagp)

        def kan_matmuls(pts, wbt, wst, bf, spl, ic, n_ic):
            for oc in range(8):
                nc.tensor.matmul(
                    pts[oc][:, :], wbt[:, oc, :], bf[:, :],
                    start=(ic == 0), stop=False)
                for p in range(4):
                    nc.tensor.matmul(
                        pts[oc][:, :], wst[:, oc, p, :, :],
                        spl[:, 2 * p:2 * p + 2, :],
                        start=False,
                        stop=(ic == n_ic - 1 and p == 3),
                        perf_mode=DR)

        # ---- Layer 0 og0: hoist silu + all 64 base matmuls to the front;
        # they only need the cheap silu features, so they fill the PE while
        # the abs/KANB spline-feature pipeline warms up.
        pts = [psum.tile([128, NB], F32, tag=f"bank{oc}", name=f"psA0_{oc}")
               for oc in range(8)]
        wbts = [None] * 8
        xts = [None] * 8
        for ic in range(8):
            xt = ipool.tile([128, NB], F16, tag=f"xt{ic}", name=f"xt{ic}")
            xts[ic] = xt
            nc.sync.dma_start(out=xt[:, :], in_=x_d[ic, :, :])
            bf = fpool.tile([128, NB], F16, tag=f"bf{ic}", name=f"bf0_{ic}")
            nc.scalar.activation(bf[:, :], xt[:, :], AF.Silu,
                                 bias=bias_ap(0.0), scale=1.0)
            bfs[ic] = bf
            wbt = wpool.tile([128, 8, 128], F16, tag="wb", bufs=8,
                             name=f"w0b_0_{ic}")
            nc.sync.dma_start(out=wbt[:, :, :], in_=w0b_d[ic, 0])
            wbts[ic] = wbt
        for ic in range(8):
            for oc in range(8):
                nc.tensor.matmul(pts[oc][:, :], wbts[ic][:, oc, :],
                                 bfs[ic][:, :], start=(ic == 0), stop=False)
        for ic in range(8):
            spls[ic] = emit_spline(xts[ic][:, :], ic, f"0_{ic}")
            wst = wpool.tile([128, 8, 4, 2, 128], F8, tag="ws", bufs=3,
                             name=f"w0s_0_{ic}")
            nc.sync.dma_start(out=wst[:, :, :, :, :], in_=w0s_d[ic, 0])
            for oc in range(8):
                for p in range(4):
                    nc.tensor.matmul(
                        pts[oc][:, :], wst[:, oc, p, :, :],
                        spls[ic][:, 2 * p:2 * p + 2, :],
                        start=False, stop=(ic == 7 and p == 3),
                        perf_mode=DR)
        for oc in range(8):
            h0t = fpool.tile([128, NB], F16, tag=f"h0_{oc}", name=f"h0t{oc}")
            nc.vector.tensor_scalar(h0t[:, :], pts[oc][:, :], sct[:, 0:1],
                                    None, ALU.mult, ALU.bypass)
            h0ts[oc] = h0t

        # ---- Layer 0 og1: spline feats already resident, no fill problem
        pts = [psum.tile([128, NB], F32, tag=f"bank{oc}", name=f"psA1_{oc}")
               for oc in range(8)]
        for ic in range(8):
            wbt = wpool.tile([128, 8, 128], F16, tag="wb", bufs=8,
                             name=f"w0b_1_{ic}")
            nc.sync.dma_start(out=wbt[:, :, :], in_=w0b_d[ic, 1])
            wst = wpool.tile([128, 8, 4, 2, 128], F8, tag="ws", bufs=3,
                             name=f"w0s_1_{ic}")
            nc.sync.dma_start(out=wst[:, :, :, :, :], in_=w0s_d[ic, 1])
            kan_matmuls(pts, wbt, wst, bfs[ic], spls[ic], ic, 8)
            # recompute feats for layer 1 while og1 matmuls drain
            bfs[ic], spls[ic] = emit_feats(h0ts[ic][:, :], ic, f"1_{ic}")
        for oc in range(8):
            h0t = fpool.tile([128, NB], F16, tag=f"h0_{8 + oc}",
                             name=f"h0t{8 + oc}")
            nc.vector.tensor_scalar(h0t[:, :], pts[oc][:, :], sct[:, 0:1],
                                    None, ALU.mult, ALU.bypass)
            h0ts[8 + oc] = h0t

        # head weights: small, load while layer 1 runs
        dwt = opool.tile([128, 8, L], F16, name="dwt")
        for ic in range(8):
            nc.sync.dma_start(out=dwt[:, ic, :], in_=dw_d[ic, :, :])
        dbt = opool.tile([128, 4], F32, name="dbt")
        nc.sync.dma_start(out=dbt[:, :], in_=db_d[:, :])

        # ---- Layer 1: 8 out chunks, 16 contraction chunks ----
        pts = [psum.tile([128, NB], F32, tag=f"bank{oc}", name=f"psB{oc}")
               for oc in range(8)]
        for ic in range(16):
            if ic >= 8:
                bfs[ic % 8], spls[ic % 8] = emit_feats(
                    h0ts[ic][:, :], ic % 8, f"1_{ic}")
            wbt = wpool.tile([128, 8, 128], F16, tag="wb", bufs=8,
                             name=f"w1b_{ic}")
            nc.sync.dma_start(out=wbt[:, :, :], in_=w1b_d[ic])
            wst = wpool.tile([128, 8, 4, 2, 128], F8, tag="ws", bufs=3,
                             name=f"w1s_{ic}")
            nc.sync.dma_start(out=wst[:, :, :, :, :], in_=w1s_d[ic])
            kan_matmuls(pts, wbt, wst, bfs[ic % 8], spls[ic % 8], ic, 16)

        # ---- Head: relu(h1) @ dw.T + db ----
        rl = opool.tile([128, 8, NB], F16, name="rl")
        for oc in range(8):
            nc.vector.tensor_scalar(rl[:, oc, :], pts[oc][:, :],
                                    sct[:, 1:2], 0.0, ALU.mult, ALU.max)
        for lc in range(4):
            pt = psum.tile([128, NB], F32, tag=f"bank{lc}", name=f"psC{lc}")
            for ic in range(8):
                nc.tensor.matmul(pt[:, :], dwt[:, ic, lc * 128:(lc + 1) * 128],
                                 rl[:, ic, :],
                                 start=(ic == 0), stop=(ic == 7))
            ot = opool.tile([128, NB], F32, tag="ot", bufs=2, name=f"ot{lc}")
            nc.vector.tensor_scalar(ot[:, :], pt[:, :], dbt[:, lc:lc + 1],
                                    None, ALU.add, ALU.bypass)
            nc.sync.dma_start(out=o_d[lc, :, :], in_=ot[:, :])

    nc.compile()
    return nc


def _pow2_scale(maxabs):
    # largest power of two keeping maxabs*S <= ~120 (half of e4m3 top 240)
    if not np.isfinite(maxabs) or maxabs <= 0:
        return 1.0
    return float(2.0 ** np.floor(np.log2(120.0 / maxabs)))


def _prep_weights(bw0, sw0, ss0, bw1, sw1, ss1, dw, db):
    out = {}
    for li, (bw, sw, ss, n_ic, n_og) in enumerate(
            ((bw0, sw0, ss0, 8, 2), (bw1, sw1, ss1, 16, 1))):
        W = -(sw * (ss[:, :, None] / 6.0))          # [out, in, 8]
        S = _pow2_scale(float(np.abs(W).max()))
        out[f"sc{li}"] = 1.0 / S
        W8 = np.clip(W * S, -240.0, 240.0).astype(ml_dtypes.float8_e4m3)
        n_out, n_in = W.shape[0], W.shape[1]
        # [out, in, k] -> [ic, og, isub, oc, p, two, osub]
        v = W8.reshape(n_og, 8, 128, n_ic, 128, 4, 2)
        ws = np.ascontiguousarray(v.transpose(3, 0, 4, 1, 5, 6, 2))
        out[f"w{li}s"] = ws.reshape((n_ic,) + ((2,) if n_og == 2 else ())
                                    + (128, 8, 4, 2, 128))
        Wb = (bw * S).astype(np.float16)
        vb = Wb.reshape(n_og, 8, 128, n_ic, 128)
        wb = np.ascontiguousarray(vb.transpose(3, 0, 4, 1, 2))
        out[f"w{li}b"] = wb.reshape((n_ic,) + ((2,) if n_og == 2 else ())
                                    + (128, 8, 128))
    dwt = np.ascontiguousarray(dw.T.reshape(8, 128, L)).astype(np.float16)
    dbt = np.ascontiguousarray(db.reshape(4, 128).T.astype(np.float32))
    sct = np.empty((128, 2), np.float32)
    sct[:, 0] = out["sc0"]
    sct[:, 1] = out["sc1"]
    return out["w0b"], out["w0s"], out["w1b"], out["w1s"], dwt, dbt, sct


def kernel(x, bw0, sw0, ss0, bw1, sw1, ss1, dw, db):
    if "nc" not in _cache:
        _cache["nc"] = _build_full()
    nc = _cache["nc"]
    w0b, w0s, w1b, w1s, dwt, dbt, sct = _prep_weights(
        np.asarray(bw0, np.float32), np.asarray(sw0, np.float32),
        np.asarray(ss0, np.float32), np.asarray(bw1, np.float32),
        np.asarray(sw1, np.float32), np.asarray(ss1, np.float32),
        np.asarray(dw, np.float32), np.asarray(db, np.float32))
    xT = np.ascontiguousarray(np.asarray(x, np.float32).T).astype(np.float16)
    in_maps = []
    for c in range(NCORES):
        xc = np.ascontiguousarray(
            xT[:, c * BC:(c + 1) * BC].reshape(8, 128, BC))
        in_maps.append({"x_d": xc, "w0b_d": w0b, "w0s_d": w0s,
                        "w1b_d": w1b, "w1s_d": w1s,
                        "dw_d": dwt, "db_d": dbt, "sc_d": sct})
    _cache["in_maps"] = in_maps
    res = run_bass_kernel_spmd(nc, in_maps, list(range(NCORES)))
    out = np.empty((B, L), np.float32)
    for c in range(NCORES):
        oc = res.results[c]["o_d"]          # (4, 128, BC)
        out[c * BC:(c + 1) * BC, :] = oc.reshape(L, BC).T
    return out
